# revision 1
# baseline (speedup 1.0000x reference)
"""NimbusLinear (VQ codebook) Trainium2 kernel.

Math: the reference's selection/threshold/sign/tree_des_mat/softmax/argmax
chain is exactly a depth-4 binary-tree threshold descent per (row, codeblock):
  node j at level l compares chosen[n, c*4+l] > thresholds[c*15+j]
  leaf index -> one-hot Encoded[n, c*16+k]
and the final einsum is a dense matmul out = Encoded @ lut_perm with
lut_perm[k*256+c, j] = lut[j, c, k].

Device strategy (8 cores, data-parallel over N rows, 512 rows/core, no
collectives):
  - encode: 15 threshold compares (exact fp32, matching the reference's
    comparisons bit-exactly) + mux-tree descent + one-hot, all on DVE in
    bf16 where values are small ints (exact)
  - big matmul on PE in a bf16 hi+lo split (Encoded is exactly 0/1 in
    bf16; lut = hi + lo rounds to ~2^-17 per element), accumulated into
    the same fp32 PSUM bank -> measured 2.3e-6 scale-relative absmax
    error vs the fp32 reference (at the fp32 accumulation noise floor).

Measured per-core time ~540us (wall-clock-delta over repeated NEFF
executions); cost-model timeline 472us, of which 436us is the PE
streaming floor for 2048 [128x128]@[128x512] bf16 matmuls. The kernel is
PE-bound; lut DMA (64MB/core) and the encode phase hide under it.

Modes (set MODE):
  - "f32r" (default): single pass with float32r matmuls (tf32-class
    multiply at full PE rate, HW-measured ~2^-13 per-product). Modeled
    266us/core; measured 1.08e-4 scale-relative absmax error.
  - "bf16x2": bf16 hi+lo split, two passes. Modeled 472us, measured
    ~540us/core wall-delta; 2.3e-6 error (fp32 noise floor). Use this if
    the correctness gate turns out stricter than ~1e-4 scale-relative.
  - "fp8hi": fp8e4m3 DoubleRow hi + bf16 lo; 477us measured, 4e-5 error.
    DoubleRow's 256-col LDWEIGHTS (no FWL) eats most of its modeled gain.
"""

import sys

sys.path.insert(0, "/opt/trn_rl_repo")

import numpy as np
import ml_dtypes

K = 16
DEPTH = 4
C = 256
IN_FEATURES = 4096
OUT_FEATURES = 4096
N_ROWS = 4096
NCORES = 8
NSH = N_ROWS // NCORES  # 512 rows per core
NCHUNK = NSH // 128  # 4 partition chunks of rows per core
CKCHUNKS = (C * K) // 128  # 32 contraction chunks
JSLABS = OUT_FEATURES // 512  # 8 output column slabs

_CACHED = {}
MODE = "f32r"  # or "bf16x2" / "fp8hi"


def _level_of_node(i):
    return int(np.floor(np.log2(i + 1)))


def _build_program(jslabs=JSLABS, encode=True, mode="bf16x2", repeats=1):
    import concourse.bacc as bacc
    import concourse.mybir as mybir
    import concourse.tile as tile
    import concourse.bass as bass

    f32 = mybir.dt.float32
    bf16 = mybir.dt.bfloat16
    fp8 = mybir.dt.float8e4

    nc = bacc.Bacc("TRN2", target_bir_lowering=False, debug=False,
                   num_devices=NCORES)

    # inputs (per-core shapes)
    xg = nc.dram_tensor("xg", [DEPTH, 2, 128, NSH], f32, kind="ExternalInput")
    th = nc.dram_tensor("th", [2, 128, 15], f32, kind="ExternalInput")
    f32r = mybir.dt.float32r
    if mode == "bf16x2":
        lhi = nc.dram_tensor("lhi", [JSLABS, 2, 128, 16, 512], bf16,
                             kind="ExternalInput")
    elif mode == "fp8hi":
        lhi8 = nc.dram_tensor("lhi8", [JSLABS, 128, 16, 2, 512], fp8,
                              kind="ExternalInput")
    if mode == "f32r":
        llo = nc.dram_tensor("lutr", [JSLABS, 2, 128, 16, 512], f32r,
                             kind="ExternalInput")
    else:
        llo = nc.dram_tensor("llo", [JSLABS, 2, 128, 16, 512], bf16,
                             kind="ExternalInput")
    out = nc.dram_tensor("out", [NCHUNK, 128, OUT_FEATURES], f32,
                         kind="ExternalOutput")

    gt = mybir.AluOpType.is_gt
    eq = mybir.AluOpType.is_equal

    with tile.TileContext(nc) as tc:
        # all pools stay open for the whole program: closing the encode pools
        # early lets the lut pool recycle their SBUF range, which makes the
        # first lut DMA inherit a WAR wait on the ENTIRE encode phase.
        with tc.tile_pool(name="enc", bufs=1) as encp, \
             tc.tile_pool(name="encwork", bufs=1) as wp, \
             tc.tile_pool(name="enctmp", bufs=1) as tp, \
             tc.tile_pool(name="lut", bufs=2) as lutp, \
             tc.tile_pool(name="ostage", bufs=4) as osp, \
             tc.tile_pool(name="psum", bufs=8,
                          space=bass.MemorySpace.PSUM) as psp:
            # one-hot Encoded^T tiles: enc[k*2+cc] rows = ck chunk k*256+cc*128
            enc_dt = f32r if mode == "f32r" else bf16
            enc = [encp.tile([128, NSH], enc_dt, tag=f"enc{i}", name=f"enc{i}")
                   for i in range(2 * K)]
            enc8 = None
            if mode == "fp8hi":
                # fp8 one-hot, (cc0, cc1) chunk pair interleaved for DoubleRow
                enc8 = [encp.tile([128, 2, NSH], fp8, tag=f"enc8_{k}",
                                  name=f"enc8_{k}") for k in range(K)]

            # ---------------- encode phase ----------------
            for _rep in range(repeats):
                for cc in range(2 if encode else 0):
                    tht = wp.tile([128, 15], f32, tag=f"th{cc}")
                    nc.sync.dma_start(tht[:], th[cc])
                    xt = [wp.tile([128, NSH], f32, tag=f"x{l}_{cc}", name=f"x{l}_{cc}")
                          for l in range(DEPTH)]
                    for l in range(DEPTH):
                        nc.sync.dma_start(xt[l][:], xg[l, cc])

                    # 15 node compares: B[i] = (x_level(i) > th_i)
                    B = [tp.tile([128, NSH], bf16, tag=f"b{i}", name=f"b{i}")
                         for i in range(15)]
                    for i in range(15):
                        nc.vector.tensor_single_scalar(
                            B[i][:], xt[_level_of_node(i)][:],
                            tht[:, i:i + 1], gt)

                    def mux(u, v, s, tag):
                        # u + s*(v-u), all values in {0,1} (exact in bf16)
                        t = tp.tile([128, NSH], bf16, tag=tag, name=f"mux_{tag}")
                        nc.vector.tensor_sub(t[:], v[:], u[:])
                        nc.vector.tensor_mul(t[:], t[:], s[:])
                        nc.vector.tensor_add(t[:], t[:], u[:])
                        return t

                    b0 = B[0]
                    b1 = mux(B[1], B[2], b0, "m1")
                    m0 = mux(B[3], B[4], b1, "m20")
                    m1 = mux(B[5], B[6], b1, "m21")
                    b2 = mux(m0, m1, b0, "m2")
                    c00 = mux(B[7], B[8], b2, "c00")
                    c01 = mux(B[9], B[10], b2, "c01")
                    c10 = mux(B[11], B[12], b2, "c10")
                    c11 = mux(B[13], B[14], b2, "c11")
                    d0 = mux(c00, c01, b1, "d0")
                    d1 = mux(c10, c11, b1, "d1")
                    b3 = mux(d0, d1, b0, "d")

                    # idx = 8*b0 + 4*b1 + 2*b2 + b3 (small ints, exact in bf16)
                    idx = tp.tile([128, NSH], bf16, tag="idx", name="idx")
                    nc.vector.tensor_scalar_mul(idx[:], b0[:], 2.0)
                    nc.vector.tensor_add(idx[:], idx[:], b1[:])
                    nc.vector.tensor_scalar_mul(idx[:], idx[:], 2.0)
                    nc.vector.tensor_add(idx[:], idx[:], b2[:])
                    nc.vector.tensor_scalar_mul(idx[:], idx[:], 2.0)
                    nc.vector.tensor_add(idx[:], idx[:], b3[:])

                    for k in range(K):
                        nc.vector.tensor_single_scalar(
                            enc[k * 2 + cc][:], idx[:], float(k), eq)
                    if mode == "fp8hi":
                        for k in range(K):
                            nc.vector.tensor_single_scalar(
                                enc8[k][:, cc, :], idx[:], float(k), eq)

                if not encode:
                    for i in range(2 * K):
                        nc.vector.memset(enc[i][:], 0.0)

                # ---------------- matmul phase ----------------
                for j in range(jslabs):
                        ps = [psp.tile([128, 512], f32, tag="ps", name=f"ps{j}_{m}")
                              for m in range(NCHUNK)]
                        for half in range(2):
                            if mode == "bf16x2":
                                hi_t = lutp.tile([128, 16, 512], bf16, tag="hi")
                                if j == 0 and half == 0:
                                    # split the very first load so the PE's
                                    # first matmuls aren't gated on one 4MB DMA
                                    for q in range(4):
                                        nc.sync.dma_start(
                                            hi_t[:, 4 * q:4 * (q + 1), :],
                                            lhi[j, half, :, 4 * q:4 * (q + 1), :])
                                else:
                                    nc.sync.dma_start(hi_t[:], lhi[j, half])
                            lo_dt = f32r if mode == "f32r" else bf16
                            lo_t = lutp.tile([128, 16, 512], lo_dt, tag="lo")
                            nc.sync.dma_start(lo_t[:], llo[j, half])
                            for kkh in range(16):
                                kk = 2 * kkh + half  # half 0 = even kk (cc=0)
                                first = kk == 0
                                last = kk == CKCHUNKS - 1
                                for m in range(NCHUNK):
                                    w = enc[kk][:, m * 128:(m + 1) * 128]
                                    if mode == "bf16x2":
                                        nc.tensor.matmul(
                                            ps[m][:], w, hi_t[:, kkh, :],
                                            start=first, stop=False)
                                        nc.tensor.matmul(
                                            ps[m][:], w, lo_t[:, kkh, :],
                                            start=False, stop=last)
                                    else:
                                        nc.tensor.matmul(
                                            ps[m][:], w, lo_t[:, kkh, :],
                                            start=first, stop=last)
                            if mode == "fp8hi" and half == 0:
                                # fp8 DoubleRow hi pass: each bucket k contracts
                                # its full 256-row (cc0,cc1) chunk pair at once
                                l8_t = lutp.tile([128, 16, 2, 512], fp8, tag="l8")
                                nc.sync.dma_start(l8_t[:], lhi8[j])
                                for k in range(K):
                                    for m in range(NCHUNK):
                                        w8 = enc8[k][:, :, m * 128:(m + 1) * 128]
                                        nc.tensor.matmul(
                                            ps[m][:], w8, l8_t[:, k, :, :],
                                            start=False, stop=False,
                                            perf_mode=mybir.MatmulPerfMode.DoubleRow)
                        for m in range(NCHUNK):
                            ot = osp.tile([128, 512], f32, tag="ot",
                                          name=f"ot{j}_{m}")
                            nc.vector.tensor_copy(ot[:], ps[m][:])
                            nc.sync.dma_start(
                                out[m, :, j * 512:(j + 1) * 512], ot[:])

    nc.compile()
    return nc


def _reference_structure_ok(selection_matrix, tree_des_mat):
    base_tree = _BASE_TREE
    sm = np.asarray(selection_matrix)
    td = np.asarray(tree_des_mat)
    if sm.shape != (C * (K - 1), C * DEPTH) or td.shape != (C * K, C * (K - 1)):
        return False
    # spot-check a few diagonal/off-diagonal blocks rather than the full
    # matrices (full check is cheap enough, do it)
    base_sel = np.zeros((K - 1, DEPTH), dtype=np.float32)
    base_sel[0, 0] = 1.0
    for i in range(1, K - 1):
        base_sel[i, int(np.log2(i + 1))] = 1.0
    exp_sm = np.zeros_like(sm)
    exp_td = np.ones_like(td)
    for i in range(C):
        exp_sm[i * (K - 1):(i + 1) * (K - 1), i * DEPTH:(i + 1) * DEPTH] = base_sel
        exp_td[i * K:(i + 1) * K, i * (K - 1):(i + 1) * (K - 1)] = base_tree
    return np.array_equal(sm, exp_sm) and np.array_equal(td, exp_td)


_BASE_TREE = np.array([
    [-1,-1,0,-1,0,0,0,-1,0,0,0,0,0,0,0],[-1,-1,0,-1,0,0,0,1,0,0,0,0,0,0,0],
    [-1,-1,0,1,0,0,0,0,-1,0,0,0,0,0,0],[-1,-1,0,1,0,0,0,0,1,0,0,0,0,0,0],
    [-1,1,0,0,-1,0,0,0,0,-1,0,0,0,0,0],[-1,1,0,0,-1,0,0,0,0,1,0,0,0,0,0],
    [-1,1,0,0,1,0,0,0,0,0,-1,0,0,0,0],[-1,1,0,0,1,0,0,0,0,0,1,0,0,0,0],
    [1,0,-1,0,0,-1,0,0,0,0,0,-1,0,0,0],[1,0,-1,0,0,-1,0,0,0,0,0,1,0,0,0],
    [1,0,-1,0,0,1,0,0,0,0,0,0,-1,0,0],[1,0,-1,0,0,1,0,0,0,0,0,0,1,0,0],
    [1,0,1,0,0,0,-1,0,0,0,0,0,0,-1,0],[1,0,1,0,0,0,-1,0,0,0,0,0,0,1,0],
    [1,0,1,0,0,0,1,0,0,0,0,0,0,0,-1],[1,0,1,0,0,0,1,0,0,0,0,0,0,0,1]],
    dtype=np.float32)


def _numpy_fallback(inputMatrix, dims, selection_matrix, thresholds,
                    tree_des_mat, lut):
    """Faithful numpy replication of the reference forward pass (slow)."""
    x = np.asarray(inputMatrix, np.float32)
    n = x.shape[0]
    c = lut.shape[1]
    chosen = x[:, np.asarray(dims).astype(np.int64)]
    subtracted = (np.asarray(selection_matrix, np.float32) @ chosen.T
                  - np.asarray(thresholds, np.float32))
    sign = np.sign(subtracted).astype(np.float32)
    tree_result = (np.asarray(tree_des_mat, np.float32) @ sign).T.reshape(n, c, K)
    index = np.argmax(tree_result, axis=2)
    onehot = np.eye(K, dtype=np.float32)[index]  # (n, c, K)
    lutm = np.asarray(lut, np.float32).transpose(1, 2, 0).reshape(c * K, -1)
    return (onehot.reshape(n, c * K) @ lutm).astype(np.float32)


def kernel(inputMatrix, dims, selection_matrix, thresholds, tree_des_mat, lut):
    inputMatrix = np.ascontiguousarray(np.asarray(inputMatrix, dtype=np.float32))
    dims_i = np.asarray(dims).astype(np.int64)
    thresholds = np.asarray(thresholds, dtype=np.float32)
    lut = np.asarray(lut, dtype=np.float32)

    if not _reference_structure_ok(selection_matrix, tree_des_mat):
        return _numpy_fallback(inputMatrix, dims_i, selection_matrix,
                               thresholds, tree_des_mat, lut)

    # ---- host prep ----
    # gathered inputs, level-major: X_all[l, c, n]
    chosen = inputMatrix[:, dims_i]                      # (N, C*DEPTH)
    X_all = np.ascontiguousarray(
        chosen.reshape(N_ROWS, C, DEPTH).transpose(2, 1, 0))  # (4, 256, N)
    th3 = np.ascontiguousarray(
        thresholds.reshape(C, K - 1).reshape(2, 128, 15))

    # lut_perm[k*256+c, j] = lut[j, c, k]; hi/lo split
    lut_perm = np.ascontiguousarray(
        lut.transpose(2, 1, 0).reshape(C * K, OUT_FEATURES))
    if MODE == "bf16x2":
        lut_hi = lut_perm.astype(ml_dtypes.bfloat16)
    elif MODE == "fp8hi":
        lut_hi8 = lut_perm.astype(ml_dtypes.float8_e4m3)
        lut_hi = lut_hi8
    if MODE != "f32r":
        lut_lo = (lut_perm - lut_hi.astype(np.float32)).astype(ml_dtypes.bfloat16)

    def dev_layout(a):
        # (4096 ck, 4096 j) -> [slab, half, p, kkh, jj] with kk = 2*kkh+half,
        # so half 0 holds the even (cc=0) chunks and half 1 the odd (cc=1)
        return np.ascontiguousarray(
            a.reshape(16, 2, 128, JSLABS, 512).transpose(3, 1, 2, 0, 4))

    if MODE == "f32r":
        llo_np = dev_layout(lut_perm)
    else:
        llo_np = dev_layout(lut_lo)
    if MODE == "bf16x2":
        lhi_np = dev_layout(lut_hi)
    elif MODE == "fp8hi":
        # [slab, p, k, ko, jj] with contraction row ck = k*256 + ko*128 + p
        lhi8_np = np.ascontiguousarray(
            lut_hi8.reshape(16, 2, 128, JSLABS, 512).transpose(3, 2, 0, 1, 4))

    from concourse.bass_utils import run_bass_kernel_spmd

    if "nc" not in _CACHED:
        _CACHED["nc"] = _build_program(mode=MODE)
    nc = _CACHED["nc"]

    in_maps = []
    for g in range(NCORES):
        xg_np = np.ascontiguousarray(
            X_all[:, :, g * NSH:(g + 1) * NSH].reshape(DEPTH, 2, 128, NSH))
        if MODE == "f32r":
            im = {"xg": xg_np, "th": th3, "lutr": llo_np}
        else:
            im = {"xg": xg_np, "th": th3, "llo": llo_np}
        if MODE == "bf16x2":
            im["lhi"] = lhi_np
        elif MODE == "fp8hi":
            im["lhi8"] = lhi8_np
        in_maps.append(im)

    res = run_bass_kernel_spmd(nc, in_maps, list(range(NCORES)))
    out = np.concatenate(
        [np.asarray(res.results[g]["out"], np.float32).reshape(NSH, OUT_FEATURES)
         for g in range(NCORES)], axis=0)
    return out



# revision 2
# speedup vs baseline: 1.8699x; 1.8699x over previous
"""NimbusLinear (VQ codebook) Trainium2 kernel.

Math: the reference's selection/threshold/sign/tree_des_mat/softmax/argmax
chain is exactly a depth-4 binary-tree threshold descent per (row, codeblock):
  node j at level l compares chosen[n, c*4+l] > thresholds[c*15+j]
  leaf index -> one-hot Encoded[n, c*16+k]
and the final einsum is a dense matmul out = Encoded @ lut_perm with
lut_perm[k*256+c, j] = lut[j, c, k].

Device strategy (8 cores, data-parallel over N rows, 512 rows/core, no
collectives):
  - encode: 15 exact-fp32 threshold compares + mux-tree descent + one-hot,
    on DVE, n-sliced into quarters (cc half x 256-row slice) so the PE can
    start consuming one-hot chunks ~15us in.
  - matmul: lut split as fp8e4m3 hi + fp8e4m3 lo (residual); both passes run
    as fp8 DoubleRow matmuls contracting 256 rows per instruction (the
    one-hot Encoded is exact in fp8).  Logical contraction rows
    ck = (2*kp+d)*256 + cc*128 + p are paired over d = k-parity so a cc half
    is usable as soon as it is encoded.
  - out written bf16 (adds ~2e-3 scale-relative error; total ~3e-3 vs the
    2e-2 gate), converted to f32 on host.

PE cost: 1024 DoubleRow matmuls x 256 cycles ~= 109us; lut DMA 32MB fp8
~= 93us; both tracks overlap with the phased schedule below.
"""

import sys

sys.path.insert(0, "/opt/trn_rl_repo")

import numpy as np
import ml_dtypes

K = 16
DEPTH = 4
C = 256
IN_FEATURES = 4096
OUT_FEATURES = 4096
N_ROWS = 4096
NCORES = 8
NSH = N_ROWS // NCORES  # 512 rows per core
NCHUNK = NSH // 128  # 4 partition chunks of rows per core
JSLABS = OUT_FEATURES // 512  # 8 output column slabs
NSLICES = 2  # encode n-slices per cc half (256 rows each)
SLICE = NSH // NSLICES
LUT_BUFS = 18  # resident lut slab-half tiles (8KB/partition each)

_CACHED = {}


def _level_of_node(i):
    return int(np.floor(np.log2(i + 1)))


def _build_program():
    import concourse.bacc as bacc
    import concourse.mybir as mybir
    import concourse.tile as tile
    import concourse.bass as bass

    f32 = mybir.dt.float32
    bf16 = mybir.dt.bfloat16
    fp8 = mybir.dt.float8e4

    nc = bacc.Bacc("TRN2", target_bir_lowering=False, debug=False,
                   num_devices=NCORES)

    # inputs (per-core shapes)
    xg = nc.dram_tensor("xg", [2, DEPTH, 128, NSH], f32, kind="ExternalInput")
    th = nc.dram_tensor("th", [2, 128, 15], f32, kind="ExternalInput")
    # l8[j, h, cc, p, kp, d, jj] = fp8 of (hi if h==0 else lo) of
    #   lut_perm[(2*kp+d)*256 + cc*128 + p, j*512 + jj]
    l8 = nc.dram_tensor("l8", [JSLABS, 2, 2, 128, 8, 2, 512], fp8,
                        kind="ExternalInput")
    out = nc.dram_tensor("out", [NCHUNK, 128, JSLABS, 512], bf16,
                         kind="ExternalOutput")

    gt = mybir.AluOpType.is_gt
    eq = mybir.AluOpType.is_equal
    DR = mybir.MatmulPerfMode.DoubleRow

    with tile.TileContext(nc) as tc:
        # keep every pool open for the whole program: early closes let later
        # pools recycle SBUF ranges and inherit WAR waits on whole phases.
        with tc.tile_pool(name="enc", bufs=1) as encp, \
             tc.tile_pool(name="encwork", bufs=1) as wp, \
             tc.tile_pool(name="enctmp", bufs=1) as tp, \
             tc.tile_pool(name="lut", bufs=LUT_BUFS) as lutp, \
             tc.tile_pool(name="ostage", bufs=12) as osp, \
             tc.tile_pool(name="psum", bufs=8,
                          space=bass.MemorySpace.PSUM) as psp:

            # ---------------- input DMAs (issue order matters) -----------
            tht = []
            xt = []
            for cc in range(2):
                t = wp.tile([128, 15], f32, tag=f"th{cc}")
                nc.sync.dma_start(t[:], th[cc])
                tht.append(t)
                row = []
                for l in range(DEPTH):
                    x = wp.tile([128, NSH], f32, tag=f"x{l}_{cc}",
                                name=f"x{l}_{cc}")
                    nc.sync.dma_start(x[:], xg[cc, l])
                    row.append(x)
                xt.append(row)

            # lut slab-half tiles; first LUT_BUFS loads fill every slot with
            # no WAR: j0-3 cc0 (8), j0-3 cc1 (8), j4 cc0 (2).
            lt = {}

            def load_lut(j, h, cc):
                t = lutp.tile([128, 8, 2, 512], fp8, tag="lut",
                              name=f"l{j}_{h}_{cc}")
                nc.sync.dma_start(t[:], l8[j, h, cc])
                lt[(j, h, cc)] = t

            for j in range(4):
                for h in range(2):
                    load_lut(j, h, 0)
            for j in range(4):
                for h in range(2):
                    load_lut(j, h, 1)
            load_lut(4, 0, 0)
            load_lut(4, 1, 0)

            # one-hot tiles enc8[(cc, s, kp)][p, d, nn]:
            #   = 1 if idx[s*256+nn, cc*128+p] == 2*kp+d
            enc8 = {}
            for cc in range(2):
                for s in range(NSLICES):
                    for kp in range(8):
                        enc8[(cc, s, kp)] = encp.tile(
                            [128, 2, SLICE], fp8, tag=f"e{cc}_{s}_{kp}",
                            name=f"e{cc}_{s}_{kp}")

            # ---------------- encode: one quarter = (cc, slice) ----------
            def encode_quarter(cc, s):
                nsl = slice(s * SLICE, (s + 1) * SLICE)
                B = [tp.tile([128, SLICE], bf16, tag=f"b{i}", name=f"b{i}_{cc}{s}")
                     for i in range(15)]
                for i in range(15):
                    nc.vector.tensor_single_scalar(
                        B[i][:], xt[cc][_level_of_node(i)][:, nsl],
                        tht[cc][:, i:i + 1], gt)

                def mux(u, v, sel, tag):
                    # u + sel*(v-u), all values in {0,1} (exact in bf16)
                    t = tp.tile([128, SLICE], bf16, tag=tag,
                                name=f"mux_{tag}_{cc}{s}")
                    nc.vector.tensor_sub(t[:], v[:], u[:])
                    nc.vector.tensor_mul(t[:], t[:], sel[:])
                    nc.vector.tensor_add(t[:], t[:], u[:])
                    return t

                b0 = B[0]
                b1 = mux(B[1], B[2], b0, "m1")
                m0 = mux(B[3], B[4], b1, "m20")
                m1 = mux(B[5], B[6], b1, "m21")
                b2 = mux(m0, m1, b0, "m2")
                c00 = mux(B[7], B[8], b2, "c00")
                c01 = mux(B[9], B[10], b2, "c01")
                c10 = mux(B[11], B[12], b2, "c10")
                c11 = mux(B[13], B[14], b2, "c11")
                d0 = mux(c00, c01, b1, "d0")
                d1 = mux(c10, c11, b1, "d1")
                b3 = mux(d0, d1, b0, "d")

                # idx = 8*b0 + 4*b1 + 2*b2 + b3 (small ints, exact in bf16)
                idx = tp.tile([128, SLICE], bf16, tag="idx", name=f"idx{cc}{s}")
                nc.vector.tensor_scalar_mul(idx[:], b0[:], 2.0)
                nc.vector.tensor_add(idx[:], idx[:], b1[:])
                nc.vector.tensor_scalar_mul(idx[:], idx[:], 2.0)
                nc.vector.tensor_add(idx[:], idx[:], b2[:])
                nc.vector.tensor_scalar_mul(idx[:], idx[:], 2.0)
                nc.vector.tensor_add(idx[:], idx[:], b3[:])

                for k in range(K):
                    nc.vector.tensor_single_scalar(
                        enc8[(cc, s, k // 2)][:, k % 2, :], idx[:],
                        float(k), eq)

            for cc, s in ((0, 0), (1, 0), (0, 1), (1, 1)):
                encode_quarter(cc, s)

            # ---------------- matmul phases ------------------------------
            ps = {}

            def mm_phase(js, ms, cc, close):
                s = ms[0] // 2
                for j in js:
                    for m in ms:
                        if not close:
                            ps[(j, m)] = psp.tile([128, 512], f32, tag="ps",
                                                  name=f"ps{j}_{m}")
                        p = ps[(j, m)]
                        woff = (m % 2) * 128
                        for kp in range(8):
                            w = enc8[(cc, s, kp)][:, :, woff:woff + 128]
                            for h in range(2):
                                nc.tensor.matmul(
                                    p[:], w, lt[(j, h, cc)][:, kp, :, :],
                                    start=(not close and kp == 0 and h == 0),
                                    stop=(close and kp == 7 and h == 1),
                                    perf_mode=DR)
                        if close:
                            ot = osp.tile([128, 512], bf16, tag="ot",
                                          name=f"ot{j}_{m}")
                            nc.scalar.copy(ot[:], p[:])
                            nc.sync.dma_start(out[m, :, j], ot[:])

            mm_phase(range(4), (0, 1), 0, False)   # P1
            mm_phase(range(4), (0, 1), 1, True)    # P2
            mm_phase(range(4), (2, 3), 0, False)   # P3
            mm_phase(range(4), (2, 3), 1, True)    # P4

            # remaining lut loads; emitted after P4's out DMAs so their WAR
            # waits (on P3/P4 readers) don't head-of-line-block the queue.
            for j in range(5, 8):
                for h in range(2):
                    load_lut(j, h, 0)
            for j in range(4, 8):
                for h in range(2):
                    load_lut(j, h, 1)

            mm_phase(range(4, 8), (0, 1), 0, False)  # P5
            mm_phase(range(4, 8), (0, 1), 1, True)   # P6
            mm_phase(range(4, 8), (2, 3), 0, False)  # P7
            mm_phase(range(4, 8), (2, 3), 1, True)   # P8

    nc.compile()
    return nc


_BASE_TREE = np.array([
    [-1,-1,0,-1,0,0,0,-1,0,0,0,0,0,0,0],[-1,-1,0,-1,0,0,0,1,0,0,0,0,0,0,0],
    [-1,-1,0,1,0,0,0,0,-1,0,0,0,0,0,0],[-1,-1,0,1,0,0,0,0,1,0,0,0,0,0,0],
    [-1,1,0,0,-1,0,0,0,0,-1,0,0,0,0,0],[-1,1,0,0,-1,0,0,0,0,1,0,0,0,0,0],
    [-1,1,0,0,1,0,0,0,0,0,-1,0,0,0,0],[-1,1,0,0,1,0,0,0,0,0,1,0,0,0,0],
    [1,0,-1,0,0,-1,0,0,0,0,0,-1,0,0,0],[1,0,-1,0,0,-1,0,0,0,0,0,1,0,0,0],
    [1,0,-1,0,0,1,0,0,0,0,0,0,-1,0,0],[1,0,-1,0,0,1,0,0,0,0,0,0,1,0,0],
    [1,0,1,0,0,0,-1,0,0,0,0,0,0,-1,0],[1,0,1,0,0,0,-1,0,0,0,0,0,0,1,0],
    [1,0,1,0,0,0,1,0,0,0,0,0,0,0,-1],[1,0,1,0,0,0,1,0,0,0,0,0,0,0,1]],
    dtype=np.float32)


def _reference_structure_ok(selection_matrix, tree_des_mat):
    sm = np.asarray(selection_matrix)
    td = np.asarray(tree_des_mat)
    if sm.shape != (C * (K - 1), C * DEPTH) or td.shape != (C * K, C * (K - 1)):
        return False
    base_sel = np.zeros((K - 1, DEPTH), dtype=np.float32)
    base_sel[0, 0] = 1.0
    for i in range(1, K - 1):
        base_sel[i, int(np.log2(i + 1))] = 1.0
    exp_sm = np.zeros_like(sm)
    exp_td = np.ones_like(td)
    for i in range(C):
        exp_sm[i * (K - 1):(i + 1) * (K - 1), i * DEPTH:(i + 1) * DEPTH] = base_sel
        exp_td[i * K:(i + 1) * K, i * (K - 1):(i + 1) * (K - 1)] = _BASE_TREE
    return np.array_equal(sm, exp_sm) and np.array_equal(td, exp_td)


def _numpy_fallback(inputMatrix, dims, selection_matrix, thresholds,
                    tree_des_mat, lut):
    """Faithful numpy replication of the reference forward pass (slow)."""
    x = np.asarray(inputMatrix, np.float32)
    n = x.shape[0]
    c = lut.shape[1]
    chosen = x[:, np.asarray(dims).astype(np.int64)]
    subtracted = (np.asarray(selection_matrix, np.float32) @ chosen.T
                  - np.asarray(thresholds, np.float32))
    sign = np.sign(subtracted).astype(np.float32)
    tree_result = (np.asarray(tree_des_mat, np.float32) @ sign).T.reshape(n, c, K)
    index = np.argmax(tree_result, axis=2)
    onehot = np.eye(K, dtype=np.float32)[index]  # (n, c, K)
    lutm = np.asarray(lut, np.float32).transpose(1, 2, 0).reshape(c * K, -1)
    return (onehot.reshape(n, c * K) @ lutm).astype(np.float32)


def kernel(inputMatrix, dims, selection_matrix, thresholds, tree_des_mat, lut):
    inputMatrix = np.ascontiguousarray(np.asarray(inputMatrix, dtype=np.float32))
    dims_i = np.asarray(dims).astype(np.int64)
    thresholds = np.asarray(thresholds, dtype=np.float32)
    lut = np.asarray(lut, dtype=np.float32)

    if not _reference_structure_ok(selection_matrix, tree_des_mat):
        return _numpy_fallback(inputMatrix, dims_i, selection_matrix,
                               thresholds, tree_des_mat, lut)

    # ---- host prep ----
    chosen = inputMatrix[:, dims_i]  # (N, C*DEPTH)
    th3 = np.ascontiguousarray(thresholds.reshape(C, K - 1).reshape(2, 128, 15))

    # lut_perm[k*256+c, j] = lut[j, c, k]; fp8 hi + fp8 lo residual
    lut_perm = np.ascontiguousarray(
        lut.transpose(2, 1, 0).reshape(C * K, OUT_FEATURES))
    lut_hi = lut_perm.astype(ml_dtypes.float8_e4m3)
    lut_lo = (lut_perm - lut_hi.astype(np.float32)).astype(ml_dtypes.float8_e4m3)

    def dev_layout(a):
        # (4096 ck, 4096 j) -> [j, cc, p, kp, d, jj], ck = (2kp+d)*256+cc*128+p
        return a.reshape(8, 2, 2, 128, JSLABS, 512).transpose(4, 2, 3, 0, 1, 5)

    l8_np = np.ascontiguousarray(
        np.stack([dev_layout(lut_hi), dev_layout(lut_lo)], axis=1))

    from concourse.bass_utils import run_bass_kernel_spmd

    if "nc" not in _CACHED:
        _CACHED["nc"] = _build_program()
    nc = _CACHED["nc"]

    in_maps = []
    for g in range(NCORES):
        ch = chosen[g * NSH:(g + 1) * NSH].reshape(NSH, 2, 128, DEPTH)
        xg_np = np.ascontiguousarray(ch.transpose(1, 3, 2, 0))  # [cc, l, p, n]
        in_maps.append({"xg": xg_np, "th": th3, "l8": l8_np})

    res = run_bass_kernel_spmd(nc, in_maps, list(range(NCORES)))
    out = np.concatenate(
        [np.asarray(res.results[g]["out"]).astype(np.float32)
         .reshape(NSH, OUT_FEATURES) for g in range(NCORES)], axis=0)
    return out
